# revision 1
# baseline (speedup 1.0000x reference)
"""Fused multi-head attention (RoPE + GQA + softmax + o_proj) on 8 Trainium2 cores.

Sharding: core c handles batch b = c//2 and query-half qh = c%2 (1024 queries).
Each core computes full K/V for its batch (keys span all 2048 positions),
attention for its 1024 queries over all 16 heads, and the output projection.

Per-core layouts (partition dim first):
  xT  [128, 16, 2048]  x^T swizzled: partition=d%128, dims (dchunk, s)   f32r
  xq  [128, 16, 1024]  same but only this core's query rows              f32r
  Q^T/K^T are built head-dim-on-partitions so QK^T and AV contract
  naturally on the partition axis; V is built [s, j] so it is the
  stationary operand of AV directly.

All matmuls run in float32r (fp32 bits, full PE rate at free-dim >= 256,
~1.5e-4 matmul error) except the final o_proj which is bf16.
"""

import sys

sys.path.insert(0, "/opt/trn_rl_repo")

import math

import numpy as np
import ml_dtypes

import concourse.bass as bass
import concourse.mybir as mybir
import concourse.tile as tile
from concourse import bacc
from concourse.bass_utils import run_bass_kernel_spmd

P = 128
B, S, HID = 4, 2048, 2048
H, HKV, D = 16, 4, 128
SQ = S // 2
DC = HID // P  # 16
KVJ = HKV * D  # 512
REP = H // HKV  # 4
ROPE_THETA = 10000.0

F32 = mybir.dt.float32
F32R = mybir.dt.float32r
BF16 = mybir.dt.bfloat16
AL = mybir.AluOpType
AF = mybir.ActivationFunctionType

_CACHE = {}


def build_nc():
    if "nc" in _CACHE:
        return _CACHE["nc"]
    nc = bacc.Bacc("TRN2", target_bir_lowering=False)

    xT = nc.dram_tensor("xT", (P, DC, S), F32R, kind="ExternalInput")
    xq = nc.dram_tensor("xq", (P, DC, SQ), F32R, kind="ExternalInput")
    wq = nc.dram_tensor("wq", (H, P, DC, P), F32R, kind="ExternalInput")
    wk = nc.dram_tensor("wk", (P, DC, KVJ), F32R, kind="ExternalInput")
    wv = nc.dram_tensor("wv", (P, DC, KVJ), F32R, kind="ExternalInput")
    wo = nc.dram_tensor("wo", (2, P, H, HID // 2), BF16, kind="ExternalInput")
    cos_q = nc.dram_tensor("cos_q", (P, SQ), F32R, kind="ExternalInput")
    sin_q = nc.dram_tensor("sin_q", (P, SQ), F32R, kind="ExternalInput")
    cos_k = nc.dram_tensor("cos_k", (P, S), F32R, kind="ExternalInput")
    sin_k = nc.dram_tensor("sin_k", (P, S), F32R, kind="ExternalInput")
    pmat = nc.dram_tensor("pmat", (P, P), F32R, kind="ExternalInput")
    ones = nc.dram_tensor("ones", (P, 1), F32R, kind="ExternalInput")
    out = nc.dram_tensor("out", (SQ, HID), F32, kind="ExternalOutput")

    with tile.TileContext(nc) as tc:
        with (
            tc.tile_pool(name="consts", bufs=1) as consts,
            tc.tile_pool(name="dram", bufs=1, space="DRAM") as dram,
            tc.tile_pool(name="kt", bufs=1) as ktp,
            tc.tile_pool(name="vt", bufs=1) as vtp,
        ):
            pm_t = consts.tile([P, P], F32R)
            nc.sync.dma_start(pm_t[:], pmat.ap())
            ones_t = consts.tile([P, 1], F32R)
            nc.sync.dma_start(ones_t[:], ones.ap())
            attd = dram.tile([P, H, SQ], BF16)
            kt = ktp.tile([P, HKV, S], F32R)
            vt = vtp.tile([P, DC, KVJ], F32R)

            if True:

                def mk_rope(work, w):
                    def rope(raw_ps, cos_sl, sin_sl, dst, pps):
                        # dst = raw*cos + (Pmat @ raw)*sin ; all [P, w]
                        q_raw_t = work.tile([P, w], F32R, tag="rp_raw", name="q_raw")
                        q_raw = q_raw_t[:]
                        nc.vector.tensor_copy(q_raw, raw_ps)
                        sw_t = pps.tile([P, w], F32, tag="rp_swap", name="rp_swap")
                        sw = sw_t[:]
                        nc.tensor.matmul(sw, lhsT=pm_t[:], rhs=q_raw, start=True, stop=True)
                        ta_t = work.tile([P, w], F32R, tag="rp_a", name="rp_a")
                        ta = ta_t[:]
                        nc.vector.tensor_tensor(ta, q_raw, cos_sl, AL.mult)
                        tb_t = work.tile([P, w], F32R, tag="rp_b", name="rp_b")
                        tb = tb_t[:]
                        nc.vector.tensor_tensor(tb, sw, sin_sl, AL.mult)
                        nc.vector.tensor_tensor(dst, ta, tb, AL.add)
                    return rope

                ST = 256
                with tc.tile_pool(name="xin", bufs=2) as xin:
                    # ---- Phase A1a: K^T (with rope) ----
                    with (
                        tc.tile_pool(name="ktab", bufs=1) as ktab,
                        tc.tile_pool(name="wkp", bufs=1) as wkp,
                        tc.tile_pool(name="ropeA", bufs=1) as ropeA,
                        tc.tile_pool(name="ppA", bufs=3, space="PSUM") as ppA,
                        tc.tile_pool(name="ppS", bufs=2, space="PSUM") as ppS,
                    ):
                        rope = mk_rope(ropeA, ST)
                        xt0 = [
                            xin.tile([P, DC // 2, ST], F32R, tag=f"xs{hf}", name=f"xt0_{hf}")
                            for hf in range(2)
                        ]
                        wk_sb = [wkp.tile([P, KVJ], F32R, tag=f"wk{dc}", name=f"wk{dc}") for dc in range(DC)]
                        wv_sb = [wkp.tile([P, KVJ], F32R, tag=f"wv{dc}", name=f"wv{dc}") for dc in range(DC)]
                        # HWDGE is FIFO per issuing engine: order by first use
                        nc.sync.dma_start(wk_sb[0][:], wk.ap()[:, 0, :])
                        nc.sync.dma_start(xt0[0][:], xT.ap()[:, 0:8, 0:ST])
                        for dc in range(1, 8):
                            nc.sync.dma_start(wk_sb[dc][:], wk.ap()[:, dc, :])
                        nc.sync.dma_start(xt0[1][:], xT.ap()[:, 8:16, 0:ST])
                        for dc in range(8, DC):
                            nc.sync.dma_start(wk_sb[dc][:], wk.ap()[:, dc, :])
                        for dc in range(DC):
                            nc.sync.dma_start(wv_sb[dc][:], wv.ap()[:, dc, :])
                        for st in range(S // ST):
                            if st == 0:
                                xt_t = xt0
                            else:
                                xt_t = [
                                    xin.tile([P, DC // 2, ST], F32R, tag=f"xs{hf}", name=f"xt{hf}")
                                    for hf in range(2)
                                ]
                                for hf in range(2):
                                    nc.sync.dma_start(
                                        xt_t[hf][:],
                                        xT.ap()[:, hf * 8:(hf + 1) * 8, st * ST:(st + 1) * ST],
                                    )
                            ck_t = ktab.tile([P, ST], F32R, tag="ckt", bufs=2, name="ck")
                            nc.sync.dma_start(ck_t[:], cos_k.ap()[:, st * ST:(st + 1) * ST])
                            sk_t = ktab.tile([P, ST], F32R, tag="skt", bufs=2, name="sk")
                            nc.sync.dma_start(sk_t[:], sin_k.ap()[:, st * ST:(st + 1) * ST])
                            for jc in range(HKV):
                                pk = ppA.tile([P, ST], F32, tag="proj")
                                for dc in range(DC):
                                    nc.tensor.matmul(
                                        pk[:],
                                        lhsT=wk_sb[dc][:, jc * P:(jc + 1) * P],
                                        rhs=xt_t[dc // 8][:, dc % 8, :],
                                        start=(dc == 0),
                                        stop=(dc == DC - 1),
                                    )
                                rope(
                                    pk[:],
                                    ck_t[:],
                                    sk_t[:],
                                    kt[:, jc, st * ST:(st + 1) * ST],
                                    ppS,
                                )
                            for si in range(ST // P):
                                sc = st * (ST // P) + si
                                pv = ppA.tile([P, KVJ], F32, tag="projv")
                                for dc in range(DC):
                                    nc.tensor.matmul(
                                        pv[:],
                                        lhsT=xt_t[dc // 8][:, dc % 8, si * P:(si + 1) * P],
                                        rhs=wv_sb[dc][:],
                                        start=(dc == 0),
                                        stop=(dc == DC - 1),
                                    )
                                nc.vector.tensor_copy(vt[:, sc, :], pv[:])

                # ---- Phase B: per head, Q-proj + rope + attention, pipelined ----
                # PSUM banks: proj1 + swap1 + scores 2x2 + av1 + den1 = 8.
                with (
                    tc.tile_pool(name="xqp", bufs=1) as xqp,
                    tc.tile_pool(name="qtab", bufs=1) as qtab,
                    tc.tile_pool(name="wqp", bufs=2) as wqp,
                    tc.tile_pool(name="ropeB", bufs=2) as ropeB,
                    tc.tile_pool(name="bwork", bufs=3) as bwork,
                    tc.tile_pool(name="ppA2", bufs=1, space="PSUM") as ppA2,
                    tc.tile_pool(name="ppS2", bufs=1, space="PSUM") as ppS2,
                    tc.tile_pool(name="ppSc", bufs=2, space="PSUM") as ppSc,
                    tc.tile_pool(name="ppAv", bufs=1, space="PSUM") as ppAv,
                    tc.tile_pool(name="ppDn", bufs=1, space="PSUM") as ppDn,
                ):
                    rope = mk_rope(ropeB, 512)

                    def start_head(h):
                        wq_t = [
                            wqp.tile([P, DC // 2, P], F32R, tag=f"wq{hf}", name=f"wq{h}_{hf}")
                            for hf in range(2)
                        ]
                        for hf in range(2):
                            nc.sync.dma_start(wq_t[hf][:], wq.ap()[h][:, hf * 8:(hf + 1) * 8, :])
                        qh_t = bwork.tile([P, SQ], F32R, tag="qhead", bufs=2, name=f"qhead{h}")
                        return wq_t, qh_t

                    xq_sb = [
                        [
                            xqp.tile([P, 4, 512], F32R, tag=f"xq{i}_{g}", name=f"xq{i}_{g}")
                            for g in range(4)
                        ]
                        for i in range(2)
                    ]
                    nc.sync.dma_start(xq_sb[0][0][:], xq.ap()[:, 0:4, 0:512])
                    wq_cur, qhead_cur = start_head(0)
                    for g in range(1, 4):
                        nc.sync.dma_start(
                            xq_sb[0][g][:], xq.ap()[:, g * 4:(g + 1) * 4, 0:512]
                        )
                    cq_t = qtab.tile([P, SQ], F32R)
                    nc.sync.dma_start(cq_t[:], cos_q.ap())
                    sq_t = qtab.tile([P, SQ], F32R)
                    nc.sync.dma_start(sq_t[:], sin_q.ap())
                    for g in range(4):
                        nc.sync.dma_start(
                            xq_sb[1][g][:], xq.ap()[:, g * 4:(g + 1) * 4, 512:SQ]
                        )
                    for st in range(2):
                        pq = ppA2.tile([P, 512], F32, tag="projq", name="pq0")
                        for dc in range(DC):
                            nc.tensor.matmul(
                                pq[:],
                                lhsT=wq_cur[dc // 8][:, dc % 8, :],
                                rhs=xq_sb[st][dc // 4][:, dc % 4, :],
                                start=(dc == 0),
                                stop=(dc == DC - 1),
                            )
                        rope(
                            pq[:],
                            cq_t[:, st * 512:(st + 1) * 512],
                            sq_t[:, st * 512:(st + 1) * 512],
                            qhead_cur[:, st * 512:(st + 1) * 512],
                            ppS2,
                        )

                    for h in range(H):
                        kv = h // REP
                        qhead = qhead_cur
                        if h + 1 < H:
                            wq_next, qhead_next = start_head(h + 1)
                        for qh in range(2):
                            st = qh
                            pq_next = None
                            if h + 1 < H:
                                pq_next = ppA2.tile([P, 512], F32, tag="projq", name=f"pq{h + 1}_{st}")
                            qtile = qhead[:, qh * 512:(qh + 1) * 512]
                            av = ppAv.tile([P, 512], F32, tag="av")
                            den = ppDn.tile([1, 512], F32, tag="den")
                            for kp in range(8):
                                sc_ps = ppSc.tile([P, 1024], F32, tag="scores")
                                for i in range(2):
                                    kc = kp * 2 + i
                                    nc.tensor.matmul(
                                        sc_ps[:, i * 512:(i + 1) * 512],
                                        lhsT=kt[:, kv, kc * P:(kc + 1) * P],
                                        rhs=qtile,
                                        start=True,
                                        stop=True,
                                    )
                                pt = bwork.tile([P, 1024], F32R, tag="pt")
                                nc.scalar.activation(pt[:], sc_ps[:], AF.Exp)
                                for i in range(2):
                                    kc = kp * 2 + i
                                    nc.tensor.matmul(
                                        av[:],
                                        lhsT=vt[:, kc, kv * P:(kv + 1) * P],
                                        rhs=pt[:, i * 512:(i + 1) * 512],
                                        start=(kc == 0),
                                        stop=(kc == DC - 1),
                                    )
                                # two-level DVE sum tree; den matmul per 4 k-chunks
                                ps2 = bwork.tile([P, 512], F32R, tag="psum2")
                                nc.vector.tensor_tensor(
                                    ps2[:], pt[:, 0:512], pt[:, 512:1024], AL.add
                                )
                                if kp % 2 == 0:
                                    ps2_prev = ps2
                                else:
                                    ps4 = bwork.tile([P, 512], F32R, tag="psum4", bufs=2)
                                    nc.vector.tensor_tensor(
                                        ps4[:], ps2_prev[:], ps2[:], AL.add
                                    )
                                    nc.tensor.matmul(
                                        den[:],
                                        lhsT=ones_t[:],
                                        rhs=ps4[:],
                                        start=(kp == 1),
                                        stop=(kp == 7),
                                    )
                                if pq_next is not None:
                                    for j in range(2):
                                        dc = kp * 2 + j
                                        nc.tensor.matmul(
                                            pq_next[:],
                                            lhsT=wq_next[dc // 8][:, dc % 8, :],
                                            rhs=xq_sb[st][dc // 4][:, dc % 4, :],
                                            start=(dc == 0),
                                            stop=(dc == DC - 1),
                                        )
                            if pq_next is not None:
                                rope(
                                    pq_next[:],
                                    cq_t[:, st * 512:(st + 1) * 512],
                                    sq_t[:, st * 512:(st + 1) * 512],
                                    qhead_next[:, st * 512:(st + 1) * 512],
                                    ppS2,
                                )
                            r_row = bwork.tile([1, 512], F32, tag="rrow")
                            nc.vector.reciprocal(r_row[:], den[:])
                            rb = bwork.tile([P, 512], F32, tag="rb", bufs=2)
                            nc.gpsimd.partition_broadcast(rb[:], r_row[:])
                            att_sb = bwork.tile([P, 512], BF16, tag="attsb", bufs=2)
                            nc.vector.tensor_tensor(att_sb[:], av[:], rb[:], AL.mult)
                            nc.sync.dma_start(
                                attd[:, h, qh * 512:(qh + 1) * 512], att_sb[:]
                            )
                        if h + 1 < H:
                            wq_cur, qhead_cur = wq_next, qhead_next

            # ---- Phase C: o_proj (bf16) ----
            with (
                tc.tile_pool(name="wop", bufs=1) as wop,
                tc.tile_pool(name="attc", bufs=2) as attcp,
                tc.tile_pool(name="outp", bufs=3) as outp,
                tc.tile_pool(name="ppO", bufs=3, space="PSUM") as ppO,
            ):
                wo_sb = [
                    [
                        wop.tile([P, H, 512], BF16, tag=f"wo{og}{ot}", name=f"wo{og}{ot}")
                        for ot in range(2)
                    ]
                    for og in range(2)
                ]
                attc0 = attcp.tile([P, H, P], BF16, tag="attc", name="attc0")
                nc.sync.dma_start(attc0[:], attd[:, :, 0:P])
                for og in range(2):
                    for ot in range(2):
                        nc.sync.dma_start(
                            wo_sb[og][ot][:],
                            wo.ap()[og][:, :, ot * 512:(ot + 1) * 512],
                        )
                for qc in range(SQ // P):
                    if qc == 0:
                        attc = attc0
                    else:
                        attc = attcp.tile([P, H, P], BF16, tag="attc", name="attc")
                        nc.sync.dma_start(attc[:], attd[:, :, qc * P:(qc + 1) * P])
                    for og in range(2):
                        out_t = outp.tile([P, HID // 2], F32, tag="outt")
                        for ot in range(2):
                            po = ppO.tile([P, 512], F32, tag="po")
                            for hc in range(H):
                                nc.tensor.matmul(
                                    po[:],
                                    lhsT=attc[:, hc, :],
                                    rhs=wo_sb[og][ot][:, hc, :],
                                    start=(hc == 0),
                                    stop=(hc == H - 1),
                                )
                            nc.vector.tensor_copy(out_t[:, ot * 512:(ot + 1) * 512], po[:])
                        nc.sync.dma_start(
                            out.ap()[qc * P:(qc + 1) * P, og * (HID // 2):(og + 1) * (HID // 2)],
                            out_t[:],
                        )

    nc.compile()
    _CACHE["nc"] = nc
    return nc


def _host_inputs(x, Wq, Wk, Wv, Wo):
    """Build the 8 per-core input maps (numpy only)."""
    f32 = np.float32
    # shared weight swizzles
    wq_sw = np.ascontiguousarray(
        Wq.reshape(DC, P, H, P).transpose(2, 1, 0, 3), dtype=f32
    )  # [jc, p, dc, j]
    wk_sw = np.ascontiguousarray(
        Wk.reshape(DC, P, KVJ).transpose(1, 0, 2), dtype=f32
    )  # [p, dc, j]
    wv_sw = np.ascontiguousarray(Wv.reshape(DC, P, KVJ).transpose(1, 0, 2), dtype=f32)
    wo2 = Wo.reshape(H * D, HID)
    wo_sw = np.stack(
        [
            np.ascontiguousarray(
                wo2[:, og * (HID // 2):(og + 1) * (HID // 2)]
                .reshape(H, P, HID // 2)
                .transpose(1, 0, 2)
            )
            for og in range(2)
        ]
    ).astype(ml_dtypes.bfloat16)  # [og, p, hc, o]

    # rope tables
    inv_ts = ROPE_THETA ** (-2.0 * np.arange(D // 2) / D)  # [64]
    inv_full = np.concatenate([inv_ts, inv_ts])  # row j uses j%64
    pos_k = np.arange(S, dtype=np.float64)
    ang_k = inv_full[:, None] * pos_k[None, :]
    cos_k = np.cos(ang_k).astype(f32)
    sin_k = np.sin(ang_k).astype(f32)
    scale = 1.0 / math.sqrt(D)

    pmat = np.zeros((P, P), f32)  # lhsT: swap[i] = -q[i+64] (i<64), +q[i-64] (i>=64)
    for i in range(64):
        pmat[i + 64, i] = -1.0
        pmat[i, i + 64] = 1.0

    ones = np.ones((P, 1), f32)

    qtabs = []
    for qh in range(2):
        pos_q = np.arange(qh * SQ, (qh + 1) * SQ, dtype=np.float64)
        ang_q = inv_full[:, None] * pos_q[None, :]
        qtabs.append(
            (
                (np.cos(ang_q) * scale).astype(f32),
                (np.sin(ang_q) * scale).astype(f32),
            )
        )

    in_maps = []
    for c in range(8):
        b, qh = c // 2, c % 2
        xTb = np.ascontiguousarray(
            x[b].T.reshape(DC, P, S).transpose(1, 0, 2), dtype=f32
        )  # [p, dc, s]
        xqb = np.ascontiguousarray(xTb[:, :, qh * SQ:(qh + 1) * SQ])
        cos_q, sin_q = qtabs[qh]
        in_maps.append(
            {
                "xT": xTb,
                "xq": xqb,
                "wq": wq_sw,
                "wk": wk_sw,
                "wv": wv_sw,
                "wo": wo_sw,
                "cos_q": cos_q,
                "sin_q": sin_q,
                "cos_k": cos_k,
                "sin_k": sin_k,
                "pmat": pmat,
                "ones": ones,
            }
        )
    return in_maps


def kernel(x, Wq, Wk, Wv, Wo, _trace=False):
    x, Wq, Wk, Wv, Wo = (np.asarray(a, dtype=np.float32) for a in (x, Wq, Wk, Wv, Wo))
    nc = build_nc()
    in_maps = _host_inputs(x, Wq, Wk, Wv, Wo)
    res = run_bass_kernel_spmd(nc, in_maps, core_ids=list(range(8)), trace=_trace)
    out = np.empty((B, S, HID), np.float32)
    for c in range(8):
        b, qh = c // 2, c % 2
        out[b, qh * SQ:(qh + 1) * SQ] = res.results[c]["out"]
    if _trace:
        kernel.last_results = res
    return out

